# revision 1
# baseline (speedup 1.0000x reference)
"""Trainium2 Bass kernel for nn_BertCounterFactCrossOpitionCompetitionTransformer.

Strategy (data-parallel over batch, 4 batches per core on 8 cores):

The reference output depends only on gate-weighted sums, and the gate is
supported on the pre-SEP "false" segment while the pair mask restricts
attention keys to the post-SEP "option" segment.  Per batch we extract the
false rows (<=256) and option rows (<=256), pad to fixed 256, and compute:

  anom   = xf @ W_anom + b_anom + row_mask          (DVE dot-products)
  e      = exp(anom); gate g = e / sum(e)
  C_t    = [Wq_t; bq_t] @ [Wk_t; bk_t]^T * inv      (host, one GEMM operand)
  G_t^T  = (C_t)^T xf_aug^T                         (PE; replaces q AND k proj)
  S_t    = G_t xo_aug^T (+ column mask via K=1 aug matmul)
  P_t    = exp(S_t)  [rep: exp(S_rep + tanh(S_con))] with rowsum Z_t
  u_t    = P_t^T (g / Z_t)                          (PE, contraction over rows)
  afv    = xf^T g ; wrv = xo^T u_rep ; wsv = xo^T u_sup
  fused^T= [afv; wrv; wsv]  -> h^T = relu(W1^T fused^T + b1)
  y      = layernorm(h W2 + b2) * ln_g + ln_b

All biases, the 1/sqrt(D) scale, and the additive -30000 segment masks are
folded into augmented matmul operands host-side; softmax max-subtraction is
skipped (scores are O(1), masked lanes underflow exp to exactly 0).
"""

import os
import numpy as np
import ml_dtypes

B, L, D = 32, 512, 768
NCORES = 8
BPC = B // NCORES          # batches per core
NF = 256                   # padded false-segment rows
NO = 256                   # padded option-segment rows
NEGM = -30000.0
INV = 1.0 / np.sqrt(D)
P = 128
BF16 = ml_dtypes.bfloat16

NKX = 7      # k-tiles over 769 (feature+bias) contraction
NKF = 19     # k-tiles over 2305 (3D+bias) contraction
NMD = 6      # m-tiles over 768
NQ = BPC * NF
NR = BPC * NF // P


def _segment_masks(x_ids, pad_idx, sep_idx):
    sep_mask = x_ids == sep_idx
    has_sep = sep_mask.any(axis=1)
    idxs = np.argmax(sep_mask.astype(np.int32), axis=1)
    valid_mask = x_ids != pad_idx
    valid_len = valid_mask.sum(axis=1)
    fallback = np.clip(valid_len // 2, 1, max(1, L - 2))
    sep_pos = np.where(has_sep, idxs, fallback)
    pos = np.arange(L)[None, :]
    false_mask = (pos < sep_pos[:, None]) & valid_mask
    option_mask = (pos > sep_pos[:, None]) & valid_mask
    return false_mask, option_mask


def _ktile(arr, nkt):
    """[K, N] -> [128, nkt, N] zero-padded so element [kt*128+p, n] -> [p, kt, n]."""
    K, N = arr.shape
    out = np.zeros((nkt * P, N), arr.dtype)
    out[:K] = arr
    return np.ascontiguousarray(out.reshape(nkt, P, N).transpose(1, 0, 2))


def _build_program(stage=99):
    import concourse.bacc as bacc
    import concourse.mybir as mybir
    import concourse.tile as tile

    fp32 = mybir.dt.float32
    bf16 = mybir.dt.bfloat16
    AF = mybir.ActivationFunctionType

    nc = bacc.Bacc("TRN2", target_bir_lowering=False, debug=False)

    di = {}
    def dram_in(name, shape, dt):
        di[name] = nc.dram_tensor(name, list(shape), dt, kind="ExternalInput")
        return di[name]

    dram_in("xfT", (P, NKX, NQ), bf16)       # [769,1024] feat-major false rows (+ones)
    dram_in("xoT", (P, NKX, NQ), bf16)       # [769,1024] feat-major option rows (+ones)
    for t in ("con", "sup", "rep"):
        dram_in(f"wc_{t}", (P, NKX, D), bf16)  # combined [769,768] inv-scaled
    dram_in("xf_r", (P, NR, D + 2), bf16)    # row-major false rows (+ones,+mask cols)
    dram_in("xo_r", (P, NR, D), bf16)        # row-major option rows
    dram_in("mo_row", (1, NQ), bf16)         # additive option-col mask per batch
    dram_in("wa_bc", (P, D + 2), bf16)       # [W_anom; b_anom; 1] partition-broadcast
    dram_in("vc_bc", (P, D + 2), bf16)       # con k-bias col of C, partition-broadcast
    dram_in("w1", (P, NKF, D), bf16)         # [2305,768] (bias row 2304)
    dram_in("w2", (P, NKX, D), bf16)         # [769,768]  (bias row 768)
    dram_in("lng", (BPC, D), fp32)
    dram_in("lnb", (BPC, D), fp32)
    y_out = nc.dram_tensor("y", [BPC, D], fp32, kind="ExternalOutput")
    dbg = None
    if stage == 0:
        dbg = nc.dram_tensor("dbg", [P, NR, 1], bf16, kind="ExternalOutput")
    elif stage == 1:
        dbg = nc.dram_tensor("dbg", [P, NKX, NQ], bf16, kind="ExternalOutput")
    elif stage == 2:
        dbg = nc.dram_tensor("dbg", [P, 2, 2], bf16, kind="ExternalOutput")
    elif stage == 3:
        dbg = nc.dram_tensor("dbg", [P, 3 * NMD, BPC], bf16,
                             kind="ExternalOutput")

    with tile.TileContext(nc) as tc:
        with (
            tc.tile_pool(name="const", bufs=1) as const,
            tc.tile_pool(name="xin", bufs=1) as xin,
            tc.tile_pool(name="wcp", bufs=2) as wcp,
            tc.tile_pool(name="wfuse", bufs=1) as wfuse,
            tc.tile_pool(name="gt", bufs=1) as gtp,
            tc.tile_pool(name="soft", bufs=4) as soft,
            tc.tile_pool(name="stats", bufs=1) as stats,
            tc.tile_pool(name="psum_big", bufs=2, space="PSUM") as psum_big,
            tc.tile_pool(name="psum_s", bufs=2, space="PSUM") as psum_s,
            tc.tile_pool(name="psum_sm", bufs=2, space="PSUM") as psum_sm,
        ):
            def load(name, shape, dt, pool=xin, tag=None, name_=None,
                     split_k=False):
                t_ = pool.tile(list(shape), dt, tag=tag or name,
                               name=name_ or f"sb_{name}")
                if split_k:
                    for k in range(shape[1]):
                        nc.sync.dma_start(out=t_[:, k, :], in_=di[name][:, k, :])
                else:
                    nc.sync.dma_start(out=t_[:], in_=di[name][:])
                return t_

            # priority DMAs: first projection's operands, k-tile granular so
            # the first matmuls can start before the full tensors land
            wc = {"con": load("wc_con", (P, NKX, D), bf16, pool=wcp,
                              tag="wc", name_="wc_con", split_k=True)}
            xfT = load("xfT", (P, NKX, NQ), bf16, split_k=True)
            xf_r = load("xf_r", (P, NR, D + 2), bf16)
            wa_bc = load("wa_bc", (P, D + 2), bf16)
            vc_bc = load("vc_bc", (P, D + 2), bf16)
            mo_row = load("mo_row", (1, NQ), bf16)
            xoT = load("xoT", (P, NKX, NQ), bf16, split_k=True)
            xo_r = load("xo_r", (P, NR, D), bf16)
            lng = load("lng", (BPC, D), fp32)
            lnb = load("lnb", (BPC, D), fp32)

            ones_row = const.tile([1, NQ], bf16, tag="ones_row")
            nc.vector.memset(ones_row[:], 1.0)
            ones_col = const.tile([P, 1], bf16, tag="ones_col")
            nc.vector.memset(ones_col[:], 1.0)
            zbias = const.tile([P, 1], fp32, tag="zbias")
            nc.vector.memset(zbias[:], 0.0)
            eps_t = const.tile([P, 1], fp32, tag="eps")
            nc.vector.memset(eps_t[:], 1e-5)

            # persistent small tiles
            anom = stats.tile([P, NR, 1], fp32, tag="anom")
            c_col = stats.tile([P, NR, 1], fp32, tag="c_col")
            junk = stats.tile([P, D + 2], fp32, tag="junk")
            e_t = stats.tile([P, NR, 1], bf16, tag="e")
            eg = stats.tile([P, NR, 1], bf16, tag="eg")
            rsg_row = stats.tile([1, BPC], bf16, tag="rsg_row")
            rsg_f32 = stats.tile([1, BPC], fp32, tag="rsg_f32")
            rsg_bc = stats.tile([P, BPC], fp32, tag="rsg_bc")
            Zs = {t: stats.tile([P, NR, 1], fp32, tag=f"Z_{t}", name=f"Z_{t}")
                  for t in ("sup", "rep")}
            tanh_c = [stats.tile([P, 2, NO], fp32, tag=f"tanh{b}",
                                 name=f"tanh{b}") for b in range(BPC)]
            u_b = [stats.tile([P, 2, 2], bf16, tag=f"u{b}", name=f"u{b}")
                   for b in range(BPC)]
            fusedT = stats.tile([P, 3 * NMD, BPC], bf16, tag="fusedT")

            def emit_gate():
                # con's k-side bias column (tanh is not shift-invariant, so
                # unlike sup/rep it can't be dropped; apply as tanh bias)
                for j in range(NR):
                    nc.vector.tensor_mul(junk[:], xf_r[:, j, :], vc_bc[:])
                    nc.vector.reduce_sum(c_col[:, j, :], junk[:],
                                         axis=mybir.AxisListType.X)
                # anom -> e -> sg -> rsg broadcast -> eg = e/sg
                for j in range(NR):
                    nc.vector.tensor_mul(junk[:], xf_r[:, j, :], wa_bc[:])
                    nc.vector.reduce_sum(anom[:, j, :], junk[:],
                                         axis=mybir.AxisListType.X)
                nc.scalar.activation(e_t[:], anom[:], AF.Exp, bias=zbias[:])
                ps_sg = psum_sm.tile([1, BPC], fp32, tag="sm", name="ps_sg")
                for b in range(BPC):
                    for j in range(2):
                        nc.tensor.matmul(ps_sg[:, b:b + 1], ones_col[:],
                                         e_t[:, 2 * b + j, :],
                                         start=(j == 0), stop=(j == 1))
                nc.vector.reciprocal(rsg_f32[:], ps_sg[:])
                nc.vector.tensor_copy(rsg_row[:], rsg_f32[:])
                ps_rb = psum_sm.tile([P, BPC], fp32, tag="sm", name="ps_rb")
                nc.tensor.matmul(ps_rb[:], ones_row[0:1, 0:P], rsg_row[:])
                nc.vector.tensor_copy(rsg_bc[:], ps_rb[:])
                for j in range(NR):
                    nc.vector.tensor_mul(eg[:, j, :], e_t[:, j, :],
                                         rsg_bc[:, j // 2:j // 2 + 1])

            def emit_proj(t, GT):
                # sup/rep skip the m=6 (k-side bias) row: it only shifts each
                # score row by a constant, which softmax cancels exactly; con
                # needs it because tanh is not shift-invariant.
                # k-outer within groups of <=3 m-tiles (6 PSUM banks) so the
                # first k-tile DMAs are consumed as they land.
                w_ = wc[t]
                for m in range(NMD):
                    ps = psum_big.tile([P, NQ], fp32, tag="big",
                                       name=f"ps_p{t}{m}")
                    for c in range(0, NQ, 512):
                        for k in range(NKX):
                            nc.tensor.matmul(
                                ps[:, c:c + 512],
                                w_[:, k, m * P:(m + 1) * P],
                                xfT[:, k, c:c + 512],
                                start=(k == 0), stop=(k == NKX - 1))
                    if m % 2 == 0:
                        nc.scalar.copy(GT[:, m, :], ps[:])
                    else:
                        nc.vector.tensor_copy(GT[:, m, :], ps[:])

            def emit_scores(t, GT, b):
                ps_s = psum_s.tile([P, 2, NO], fp32, tag="s",
                                   name=f"ps_s{t}{b}")
                for jl in range(2):
                    q0 = b * NF + jl * P
                    for k in range(NMD):
                        nc.tensor.matmul(
                            ps_s[:, jl, :], GT[:, k, q0:q0 + P],
                            xoT[:, k, b * NO:(b + 1) * NO],
                            start=(k == 0), stop=(t == "con" and k == NMD - 1))
                    if t != "con":
                        nc.tensor.matmul(
                            ps_s[:, jl, :], ones_row[0:1, q0:q0 + P],
                            mo_row[0:1, b * NO:(b + 1) * NO],
                            start=False, stop=True)
                return ps_s

            def emit_exp(t, b, ps_s):
                if t == "con":
                    for jl in range(2):
                        nc.scalar.activation(tanh_c[b][:, jl, :],
                                             ps_s[:, jl, :], AF.Tanh,
                                             bias=c_col[:, 2 * b + jl, :])
                    return None
                if t == "rep":
                    a_t = soft.tile([P, 2, NO], fp32, tag="A", name=f"A{b}")
                    nc.vector.tensor_add(a_t[:], ps_s[:], tanh_c[b][:])
                    src = a_t
                else:
                    src = ps_s
                p_t = soft.tile([P, 2, NO], bf16, tag="P", name=f"P{t}{b}")
                for jl in range(2):
                    nc.scalar.activation(p_t[:, jl, :], src[:, jl, :],
                                         AF.Exp, bias=zbias[:],
                                         accum_out=Zs[t][:, 2 * b + jl, :])
                return p_t

            def emit_u(t, b, p_t):
                rz = soft.tile([P, 2, 1], fp32, tag="rz", name=f"rz{t}{b}")
                nc.vector.reciprocal(rz[:], Zs[t][:, 2 * b:2 * b + 2, :])
                w_t = soft.tile([P, 2, 1], bf16, tag="w", name=f"w{t}{b}")
                nc.vector.tensor_mul(w_t[:], eg[:, 2 * b:2 * b + 2, :], rz[:])
                ps_u = psum_sm.tile([P, 2, 1], fp32, tag="sm",
                                    name=f"ps_u{t}{b}")
                for mo_t in range(2):
                    for jl in range(2):
                        nc.tensor.matmul(
                            ps_u[:, mo_t, :],
                            p_t[:, jl, mo_t * P:(mo_t + 1) * P],
                            w_t[:, jl, :],
                            start=(jl == 0), stop=(jl == 1))
                tcol = 0 if t == "rep" else 1
                nc.vector.tensor_copy(u_b[b][:, :, tcol:tcol + 1], ps_u[:])

            def emit_vec(b):
                ps_a = psum_sm.tile([P, NMD, 1], fp32, tag="sm",
                                    name=f"ps_a{b}")
                ps_w = psum_sm.tile([P, NMD, 2], fp32, tag="sm",
                                    name=f"ps_w{b}")
                for mj in range(NMD):
                    for jl in range(2):
                        nc.tensor.matmul(
                            ps_a[:, mj, :],
                            xf_r[:, 2 * b + jl, mj * P:(mj + 1) * P],
                            eg[:, 2 * b + jl, :],
                            start=(jl == 0), stop=(jl == 1))
                        nc.tensor.matmul(
                            ps_w[:, mj, :],
                            xo_r[:, 2 * b + jl, mj * P:(mj + 1) * P],
                            u_b[b][:, jl, :],
                            start=(jl == 0), stop=(jl == 1))
                nc.scalar.copy(fusedT[:, 0:NMD, b:b + 1], ps_a[:])
                nc.vector.tensor_copy(fusedT[:, NMD:2 * NMD, b:b + 1],
                                      ps_w[:, :, 0:1])
                nc.vector.tensor_copy(fusedT[:, 2 * NMD:3 * NMD, b:b + 1],
                                      ps_w[:, :, 1:2])

            # ---- emission: proj_con first (PE busy ASAP), gate during it;
            # next type's projection emitted before previous type's u-matmuls
            # so PE never waits on the exp->w vector chain ----
            GT_con = gtp.tile([P, NKX, NQ], bf16, tag="GT", name="GT_con")
            emit_proj("con", GT_con)
            emit_gate()
            if stage == 0:
                nc.sync.dma_start(out=dbg[:], in_=eg[:])
            if stage >= 1:
                for b in range(BPC):
                    emit_exp("con", b, emit_scores("con", GT_con, b))

                wc["sup"] = load("wc_sup", (P, NKX, D), bf16, pool=wcp,
                                 tag="wc", name_="wc_sup")
                GT_sup = gtp.tile([P, NKX, NQ], bf16, tag="GT", name="GT_sup")
                emit_proj("sup", GT_sup)
                pts_sup = [emit_exp("sup", b, emit_scores("sup", GT_sup, b))
                           for b in range(BPC)]

                wc["rep"] = load("wc_rep", (P, NKX, D), bf16, pool=wcp,
                                 tag="wc", name_="wc_rep")
                GT_rep = gtp.tile([P, NKX, NQ], bf16, tag="GT", name="GT_rep")
                emit_proj("rep", GT_rep)
                if stage == 1:
                    nc.sync.dma_start(out=dbg[:], in_=GT_rep[:])
                for b in range(BPC):
                    emit_u("sup", b, pts_sup[b])
                if stage >= 2:
                    pts_rep = [emit_exp("rep", b,
                                        emit_scores("rep", GT_rep, b))
                               for b in range(BPC)]
                    for b in range(BPC):
                        emit_u("rep", b, pts_rep[b])
                        if stage >= 3:
                            emit_vec(b)

            if stage == 2:
                nc.sync.dma_start(out=dbg[:], in_=u_b[0][:])

            # ---- fuse MLP ----
            if stage == 3:
                nc.sync.dma_start(out=dbg[:], in_=fusedT[:])
            w1 = load("w1", (P, NKF, D), bf16, pool=wfuse, tag="w1")
            w2 = load("w2", (P, NKX, D), bf16, pool=wfuse, tag="w2")
            hT = stats.tile([P, NMD, BPC], bf16, tag="hT")
            for mh in range(NMD if stage >= 4 else 0):
                ps_h = psum_sm.tile([P, BPC], fp32, tag="sm", name=f"ps_h{mh}")
                for k in range(NKF - 1):
                    nc.tensor.matmul(ps_h[:], w1[:, k, mh * P:(mh + 1) * P],
                                     fusedT[:, k, :],
                                     start=(k == 0), stop=False)
                nc.tensor.matmul(ps_h[:], w1[0:1, NKF - 1, mh * P:(mh + 1) * P],
                                 ones_row[0:1, 0:BPC], start=False, stop=True)
                nc.scalar.activation(hT[:, mh, :], ps_h[:], AF.Relu,
                                     bias=zbias[:])
                if mh == NMD - 1:
                    # preload the Sqrt ACT LUT during fuse2's matmuls so the
                    # layernorm tail doesn't pay the ~1.3us table load
                    nc.scalar.activation(junk[0:1, 0:1], eps_t[0:1, :],
                                         AF.Sqrt, bias=eps_t[0:1, :])

            if stage >= 4:
                ps_y = psum_big.tile([BPC, D], fp32, tag="big", name="ps_y")
                mu_p = stats.tile([BPC, 2], fp32, tag="mu_p")
                for ci, c in enumerate(range(0, D, 512)):
                    cn = min(512, D - c)
                    for k in range(NMD):
                        nc.tensor.matmul(ps_y[:, c:c + cn], hT[:, k, :],
                                         w2[:, k, c:c + cn],
                                         start=(k == 0), stop=False)
                    nc.tensor.matmul(ps_y[:, c:c + cn], ones_row[0:1, 0:BPC],
                                     w2[0:1, NMD, c:c + cn],
                                     start=False, stop=True)
                    # partial mean overlaps the next chunk's matmuls
                    nc.vector.reduce_sum(mu_p[:, ci:ci + 1], ps_y[:, c:c + cn],
                                         axis=mybir.AxisListType.X)

                # ---- layernorm (fused ops to shorten the serial tail) ----
                mu = stats.tile([BPC, 1], fp32, tag="mu")
                nc.vector.reduce_sum(mu[:], mu_p[:], axis=mybir.AxisListType.X)
                nc.vector.tensor_scalar_mul(mu[:], mu[:], 1.0 / D)
                xc = stats.tile([BPC, D], fp32, tag="xc")
                nc.vector.tensor_scalar(xc[:], ps_y[:], mu[:], None,
                                        op0=mybir.AluOpType.subtract)
                var = stats.tile([BPC, 1], fp32, tag="var")
                junk2 = stats.tile([BPC, D], fp32, tag="junk2")
                nc.scalar.activation(junk2[:], xc[:], AF.Square,
                                     bias=zbias[0:BPC, :], accum_out=var[:])
                sd = stats.tile([BPC, 1], fp32, tag="sd")
                nc.scalar.activation(sd[:], var[:], AF.Sqrt, scale=1.0 / D,
                                     bias=eps_t[0:BPC, :])
                rstd = stats.tile([BPC, 1], fp32, tag="rstd")
                nc.vector.reciprocal(rstd[:], sd[:])
                yt = stats.tile([BPC, D], fp32, tag="yt")
                nc.vector.scalar_tensor_tensor(
                    yt[:], xc[:], rstd[:], lng[:],
                    op0=mybir.AluOpType.mult, op1=mybir.AluOpType.mult)
                nc.vector.tensor_add(yt[:], yt[:], lnb[:])
                nc.sync.dma_start(out=y_out[:], in_=yt[:])

    nc.compile()
    return nc


def _prep_core_inputs(x, x_ids, pad_idx, sep_idx, weights):
    """Host-side slicing/padding/layout for all cores. Returns in_maps list."""
    false_mask, option_mask = _segment_masks(x_ids, pad_idx, sep_idx)

    (W_anom, b_anom, Wq, bq, Wk, bk, W_fuse1, b_fuse1,
     W_fuse2, b_fuse2, ln_g, ln_b) = weights

    # ---- shared (weight) arrays ----
    shared = {}
    for t in ("sup", "con", "rep"):
        cq = np.concatenate([Wq[t], bq[t][None, :]], axis=0)  # [769, 768]
        ck = np.concatenate([Wk[t], bk[t][None, :]], axis=0)
        C = (cq @ ck.T) * INV                                  # [769, 769]
        shared[f"wc_{t}"] = _ktile(C[:, :D].astype(BF16), NKX)
        if t == "con":
            # con's k-bias column, applied as a per-row tanh bias on-device
            vc = np.zeros((P, D + 2), np.float32)
            vc[:, :D + 1] = C[:, D][None, :]
            shared["vc_bc"] = vc.astype(BF16)
    a1 = np.zeros((3 * D + 1, D), np.float32)
    a1[:3 * D] = W_fuse1
    a1[3 * D] = b_fuse1
    shared["w1"] = _ktile(a1.astype(BF16), NKF)
    a2 = np.zeros((D + 1, D), np.float32)
    a2[:D] = W_fuse2
    a2[D] = b_fuse2
    shared["w2"] = _ktile(a2.astype(BF16), NKX)
    wa = np.zeros((P, D + 2), np.float32)
    wa[:, :D] = W_anom[:, 0][None, :]
    wa[:, D] = b_anom[0]
    wa[:, D + 1] = 1.0
    shared["wa_bc"] = wa.astype(BF16)
    shared["lng"] = np.ascontiguousarray(
        np.broadcast_to(ln_g[None, :], (BPC, D)).astype(np.float32))
    shared["lnb"] = np.ascontiguousarray(
        np.broadcast_to(ln_b[None, :], (BPC, D)).astype(np.float32))

    in_maps = []
    for c in range(NCORES):
        xf = np.zeros((BPC, NF, D), np.float32)
        xo = np.zeros((BPC, NO, D), np.float32)
        mf = np.zeros((BPC, NF), np.float32)
        mo = np.zeros((BPC, NO), np.float32)
        for i in range(BPC):
            gb = c * BPC + i
            f_idx = np.where(false_mask[gb])[0]
            o_idx = np.where(option_mask[gb])[0]
            xf[i, :len(f_idx)] = x[gb, f_idx]
            xo[i, :len(o_idx)] = x[gb, o_idx]
            mf[i, len(f_idx):] = NEGM
            mo[i, len(o_idx):] = NEGM

        # feature-major + ones row: [769, BPC*NF]
        xfT = np.zeros((D + 1, BPC * NF), np.float32)
        xfT[:D] = xf.transpose(2, 0, 1).reshape(D, BPC * NF)
        xfT[D] = 1.0
        xoT = np.zeros((D + 1, BPC * NO), np.float32)
        xoT[:D] = xo.transpose(2, 0, 1).reshape(D, BPC * NO)
        xoT[D] = 1.0

        # row-major false rows + ones col + mask col: [1024, 770]
        xfr = np.zeros((BPC * NF, D + 2), np.float32)
        xfr[:, :D] = xf.reshape(BPC * NF, D)
        xfr[:, D] = 1.0
        xfr[:, D + 1] = mf.reshape(-1)
        xor_ = xo.reshape(BPC * NO, D)

        m = dict(shared)
        m["xfT"] = _ktile(xfT.astype(BF16), NKX)
        m["xoT"] = _ktile(xoT.astype(BF16), NKX)
        m["xf_r"] = np.ascontiguousarray(
            xfr.astype(BF16).reshape(NR, P, D + 2).transpose(1, 0, 2))
        m["xo_r"] = np.ascontiguousarray(
            xor_.astype(BF16).reshape(NR, P, D).transpose(1, 0, 2))
        m["mo_row"] = mo.reshape(1, BPC * NO).astype(BF16)
        in_maps.append(m)
    return in_maps


_CACHED_NC = None
LAST_RESULTS = None


def kernel(x, x_ids, pad_idx, sep_idx,
           W_anom, b_anom,
           Wq_sup, bq_sup, Wk_sup, bk_sup,
           Wq_con, bq_con, Wk_con, bk_con,
           Wq_rep, bq_rep, Wk_rep, bk_rep,
           W_fuse1, b_fuse1, W_fuse2, b_fuse2,
           ln_g, ln_b):
    from concourse import bass_utils

    global _CACHED_NC, LAST_RESULTS
    x = np.asarray(x, np.float32)
    x_ids = np.asarray(x_ids)
    pad_idx = int(np.asarray(pad_idx))
    sep_idx = int(np.asarray(sep_idx))
    weights = (
        np.asarray(W_anom, np.float32), np.asarray(b_anom, np.float32),
        {"sup": np.asarray(Wq_sup, np.float32),
         "con": np.asarray(Wq_con, np.float32),
         "rep": np.asarray(Wq_rep, np.float32)},
        {"sup": np.asarray(bq_sup, np.float32),
         "con": np.asarray(bq_con, np.float32),
         "rep": np.asarray(bq_rep, np.float32)},
        {"sup": np.asarray(Wk_sup, np.float32),
         "con": np.asarray(Wk_con, np.float32),
         "rep": np.asarray(Wk_rep, np.float32)},
        {"sup": np.asarray(bk_sup, np.float32),
         "con": np.asarray(bk_con, np.float32),
         "rep": np.asarray(bk_rep, np.float32)},
        np.asarray(W_fuse1, np.float32), np.asarray(b_fuse1, np.float32),
        np.asarray(W_fuse2, np.float32), np.asarray(b_fuse2, np.float32),
        np.asarray(ln_g, np.float32), np.asarray(ln_b, np.float32),
    )

    in_maps = _prep_core_inputs(x, x_ids, pad_idx, sep_idx, weights)
    if _CACHED_NC is None:
        _CACHED_NC = _build_program()
    last_err = None
    for attempt in range(3):
        try:
            res = bass_utils.run_bass_kernel_spmd(
                _CACHED_NC, in_maps, list(range(NCORES)))
            break
        except Exception as err:  # transient device-unrecoverable states
            last_err = err
            import time
            time.sleep(5 * (attempt + 1))
            try:  # re-create the PJRT client before retrying
                import jax.extend
                jax.extend.backend.clear_backends()
            except Exception:
                pass
    else:
        raise last_err
    LAST_RESULTS = res
    out = np.zeros((B, D), np.float32)
    for c in range(NCORES):
        out[c * BPC:(c + 1) * BPC] = res.results[c]["y"]
    return out



# revision 7
# speedup vs baseline: 1.1547x; 1.1547x over previous
"""Trainium2 Bass kernel for nn_BertCounterFactCrossOpitionCompetitionTransformer.

v2 — fp8 DoubleRow + host-side gate/bias precompute.

Data-parallel over batch (4 batches/core on 8 cores). Per batch the false
(pre-SEP) rows are queries, option (post-SEP) rows are keys; both padded
to 256. Device pipeline per core (q rows packed NQ=4x256=1024):

  G_t   = xf8 @ C8_t            fp8 DoubleRow matmuls (C = Wq_aug Wk_aug^T
                                 * inv, host-combined; q-bias row folded
                                 into per-column score bias rows)
  S_t   = G8_t @ xo8^T + brow   fp8 DoubleRow, per-batch 256x256 blocks
  con: tanh_c = tanh(S/SS + c_col)   (c_col = k-bias per q-row, host)
  sup: P = exp(S/SS), rep: P = exp(S/SS + tanh_c); Z row-sums via accum
  u_t   = P_t^T (g/Z)           g = gate softmax, computed exactly on host
  wrv/wsv = xo^T u_t            bf16
  h     = relu(hinit + W1[wrv;wsv] part)   (afv part + bias in hinit, host)
  y     = layernorm(h W2 + b2) * ln_g + ln_b

All fp8 scales are powers of two; score descale happens inside the
exp/tanh activations (scale operand). Host precomputes (exact fp32):
gate g, con's c_col, per-column q-bias/mask rows, afv, hinit.
"""

import numpy as np
import ml_dtypes

B, L, D = 32, 512, 768
NCORES = 8
BPC = B // NCORES
NF = 256
NO = 256
NEGM = -30000.0
INV = 1.0 / np.sqrt(D)
P = 128
BF16 = ml_dtypes.bfloat16
F8 = ml_dtypes.float8_e4m3

NKT = 6                 # feature k-tiles (768 = 6*128)
NKP = 3                 # DoubleRow k-tile pairs
NMD = 6                 # m-tiles over 768
NQ = BPC * NF           # 1024 query rows per core
NR = NQ // P            # 8 row-tiles
W1KT = 12               # [wrv; wsv] k-tiles (1536 = 12*128)
SG = 1024.0             # fp8 scale of G


def _segment_masks(x_ids, pad_idx, sep_idx):
    sep_mask = x_ids == sep_idx
    has_sep = sep_mask.any(axis=1)
    idxs = np.argmax(sep_mask.astype(np.int32), axis=1)
    valid_mask = x_ids != pad_idx
    valid_len = valid_mask.sum(axis=1)
    fallback = np.clip(valid_len // 2, 1, max(1, L - 2))
    sep_pos = np.where(has_sep, idxs, fallback)
    pos = np.arange(L)[None, :]
    false_mask = (pos < sep_pos[:, None]) & valid_mask
    option_mask = (pos > sep_pos[:, None]) & valid_mask
    return false_mask, option_mask


def _pow2_scale(absmax, target=224.0):
    return 2.0 ** np.floor(np.log2(target / max(absmax, 1e-30)))


def _build_program():
    import concourse.bacc as bacc
    import concourse.mybir as mybir
    import concourse.tile as tile

    fp32 = mybir.dt.float32
    bf16 = mybir.dt.bfloat16
    fp8 = mybir.dt.float8e4
    AF = mybir.ActivationFunctionType
    DR = mybir.MatmulPerfMode.DoubleRow
    AX = mybir.AxisListType.X

    nc = bacc.Bacc("TRN2", target_bir_lowering=False, debug=False)

    di = {}
    def dram_in(name, shape, dt):
        di[name] = nc.dram_tensor(name, list(shape), dt, kind="ExternalInput")
        return di[name]

    dram_in("xfT8", (P, NKT, NQ), fp8)     # feat-major false rows * SX
    dram_in("xoT8", (P, NKT, NQ), fp8)     # feat-major option rows * SX
    for t in ("con", "sup", "rep"):
        dram_in(f"wc_{t}", (P, NKT, D), fp8)   # C[:768,:768] * sc_t
    dram_in("xo_r", (P, NR, D), bf16)      # row-major option rows
    dram_in("g_col", (P, NR, 1), fp32)     # gate (host softmax)
    dram_in("ccol", (P, NR, 1), fp32)      # con k-bias per q-row
    dram_in("mrow_sup", (1, BPC * 2 * NO), bf16)  # (qbias+mask)*SS, dup x2
    dram_in("mrow_rep", (1, BPC * 2 * NO), bf16)
    dram_in("brow_con", (1, BPC * 2 * NO), bf16)  # qbias*SS, dup x2
    dram_in("w1", (P, W1KT, D), bf16)      # W_fuse1[768:2304] ([wrv; wsv])
    dram_in("w2", (P, NKT, D), bf16)       # W_fuse2
    dram_in("b2row", (1, D), bf16)
    dram_in("hinitT", (BPC, D), bf16)      # afv@W1[:768]+b1 (host)
    dram_in("eye4", (BPC, BPC), bf16)
    dram_in("lng", (BPC, D), fp32)
    dram_in("lnb", (BPC, D), fp32)
    y_out = nc.dram_tensor("y", [BPC, D], fp32, kind="ExternalOutput")

    with tile.TileContext(nc) as tc:
        with (
            tc.tile_pool(name="const", bufs=1) as const,
            tc.tile_pool(name="xin", bufs=1) as xin,
            tc.tile_pool(name="gt", bufs=1) as gtp,
            tc.tile_pool(name="soft", bufs=2) as soft,
            tc.tile_pool(name="stats", bufs=1) as stats,
            tc.tile_pool(name="psum_big", bufs=2, space="PSUM") as psum_big,
            tc.tile_pool(name="psum_s", bufs=2, space="PSUM") as psum_s,
            tc.tile_pool(name="psum_sm", bufs=2, space="PSUM") as psum_sm,
        ):
            def load(name, shape, dt, chunks=None):
                t_ = xin.tile(list(shape), dt, tag=name, name=f"sb_{name}")
                if chunks is None:
                    nc.sync.dma_start(out=t_[:], in_=di[name][:])
                else:
                    for sl in chunks:
                        nc.sync.dma_start(out=t_[(slice(None),) + sl],
                                          in_=di[name][(slice(None),) + sl])
                return t_

            # ---- DMA priority order ----
            # proj_con first: wc_con m-slices + xfT c-halves
            m_slices = [(slice(None), slice(m * P, (m + 1) * P))
                        for m in range(NMD)]
            c_halves = [(slice(None), slice(c * 512, (c + 1) * 512))
                        for c in range(2)]
            wc = {"con": load("wc_con", (P, NKT, D), fp8, chunks=m_slices)}
            xfT8 = load("xfT8", (P, NKT, NQ), fp8, chunks=c_halves)
            wc["sup"] = load("wc_sup", (P, NKT, D), fp8, chunks=m_slices)
            xoT8 = load("xoT8", (P, NKT, NQ), fp8, chunks=c_halves)
            wc["rep"] = load("wc_rep", (P, NKT, D), fp8, chunks=m_slices)
            # smalls needed mid-pipeline
            g_col = load("g_col", (P, NR, 1), fp32)
            ccol = load("ccol", (P, NR, 1), fp32)
            mrow = {"sup": load("mrow_sup", (1, BPC * 2 * NO), bf16),
                    "rep": load("mrow_rep", (1, BPC * 2 * NO), bf16)}
            brow_con = load("brow_con", (1, BPC * 2 * NO), bf16)
            xo_r = load("xo_r", (P, NR, D), bf16,
                        chunks=[(slice(j, j + 1), slice(None))
                                for j in range(NR)])
            hinitT = load("hinitT", (BPC, D), bf16)
            eye4 = load("eye4", (BPC, BPC), bf16)
            lng = load("lng", (BPC, D), fp32)
            lnb = load("lnb", (BPC, D), fp32)
            w1 = load("w1", (P, W1KT, D), bf16,
                      chunks=[(slice(k, k + 1), slice(None))
                              for k in range(W1KT)])
            w2 = load("w2", (P, NKT, D), bf16)
            b2row = load("b2row", (1, D), bf16)

            ones_row = const.tile([1, P], bf16, tag="ones_row")
            nc.vector.memset(ones_row[:], 1.0)
            zbias = const.tile([P, 1], fp32, tag="zbias")
            nc.vector.memset(zbias[:], 0.0)
            eps_t = const.tile([P, 1], fp32, tag="eps")
            nc.vector.memset(eps_t[:], 1e-5)

            # persistent tiles
            GT8 = {t: gtp.tile([P, NKT, NQ], fp8, tag=f"GT_{t}",
                               name=f"GT_{t}") for t in ("con", "sup", "rep")}
            Zs = {t: stats.tile([P, NR, 1], fp32, tag=f"Z_{t}", name=f"Z_{t}")
                  for t in ("sup", "rep")}
            tanh_c = [stats.tile([P, 2, NO], fp32, tag=f"tanh{b}",
                                 name=f"tanh{b}") for b in range(BPC)]
            pts_sup = [stats.tile([P, 2, NO], bf16, tag=f"psup{b}",
                                  name=f"psup{b}") for b in range(BPC)]
            u_b = [stats.tile([P, 2, 2], bf16, tag=f"u{b}", name=f"u{b}")
                   for b in range(BPC)]
            fusedT = stats.tile([P, W1KT, BPC], bf16, tag="fusedT")
            hT = stats.tile([P, NMD, BPC], bf16, tag="hT")

            def emit_proj(t, gdescale):
                w_ = wc[t]
                for m in range(NMD):
                    ps = psum_big.tile([P, NQ], fp32, tag="big",
                                       name=f"ps_p{t}{m}")
                    for c in range(2):
                        for k in range(NKP):
                            nc.tensor.matmul(
                                ps[:, c * 512:(c + 1) * 512],
                                w_[:, 2 * k:2 * k + 2, m * P:(m + 1) * P],
                                xfT8[:, 2 * k:2 * k + 2,
                                     c * 512:(c + 1) * 512],
                                start=(k == 0), stop=(k == NKP - 1),
                                perf_mode=DR)
                    # fp8 quantize G with pow2 rescale; alternate engines
                    if m % 2 == 0:
                        nc.vector.tensor_scalar_mul(GT8[t][:, m, :], ps[:],
                                                    gdescale)
                    else:
                        nc.scalar.activation(GT8[t][:, m, :], ps[:], AF.Copy,
                                             scale=gdescale, bias=0.0)

            def emit_scores(t, b):
                ps_s = psum_s.tile([P, 2, NO], fp32, tag="s",
                                   name=f"ps_s{t}{b}")
                brow = brow_con if t == "con" else mrow[t]
                for jl in range(2):
                    q0 = b * NF + jl * P
                    for k in range(NKP):
                        nc.tensor.matmul(
                            ps_s[:, jl, :],
                            GT8[t][:, 2 * k:2 * k + 2, q0:q0 + P],
                            xoT8[:, 2 * k:2 * k + 2, b * NO:(b + 1) * NO],
                            start=(k == 0), stop=False, perf_mode=DR)
                    nc.tensor.matmul(
                        ps_s[:, jl, :], ones_row[0:1, 0:P],
                        brow[0:1, (b * 2 + jl) * NO:(b * 2 + jl + 1) * NO],
                        start=False, stop=True)
                return ps_s

            def emit_exp(t, b, ps_s, inv_ss):
                if t == "con":
                    for jl in range(2):
                        nc.scalar.activation(tanh_c[b][:, jl, :],
                                             ps_s[:, jl, :], AF.Tanh,
                                             scale=inv_ss,
                                             bias=ccol[:, 2 * b + jl, :])
                    return None
                if t == "rep":
                    a_t = soft.tile([P, 2, NO], fp32, tag="A", name=f"A{b}")
                    nc.vector.scalar_tensor_tensor(
                        a_t[:], ps_s[:], inv_ss, tanh_c[b][:],
                        op0=mybir.AluOpType.mult, op1=mybir.AluOpType.add)
                    p_t = soft.tile([P, 2, NO], bf16, tag="Pr", name=f"Pr{b}")
                    for jl in range(2):
                        nc.scalar.activation(p_t[:, jl, :], a_t[:, jl, :],
                                             AF.Exp, bias=zbias[:],
                                             accum_out=Zs[t][:, 2 * b + jl, :])
                    return p_t
                p_t = pts_sup[b]
                for jl in range(2):
                    nc.scalar.activation(p_t[:, jl, :], ps_s[:, jl, :],
                                         AF.Exp, scale=inv_ss, bias=zbias[:],
                                         accum_out=Zs[t][:, 2 * b + jl, :])
                return p_t

            def emit_u(t, b, p_t):
                rz = soft.tile([P, 2, 1], fp32, tag="rz", name=f"rz{t}{b}")
                nc.vector.reciprocal(rz[:], Zs[t][:, 2 * b:2 * b + 2, :])
                w_t = soft.tile([P, 2, 1], bf16, tag="w", name=f"w{t}{b}")
                nc.vector.tensor_mul(w_t[:], g_col[:, 2 * b:2 * b + 2, :],
                                     rz[:])
                ps_u = psum_sm.tile([P, 2, 1], fp32, tag="sm",
                                    name=f"ps_u{t}{b}")
                for mo_t in range(2):
                    for jl in range(2):
                        nc.tensor.matmul(
                            ps_u[:, mo_t, :],
                            p_t[:, jl, mo_t * P:(mo_t + 1) * P],
                            w_t[:, jl, :],
                            start=(jl == 0), stop=(jl == 1))
                tcol = 0 if t == "rep" else 1
                nc.vector.tensor_copy(u_b[b][:, :, tcol:tcol + 1], ps_u[:])

            def emit_vec(b):
                ps_w = psum_sm.tile([P, NMD, 2], fp32, tag="sm",
                                    name=f"ps_w{b}")
                for mj in range(NMD):
                    for jl in range(2):
                        nc.tensor.matmul(
                            ps_w[:, mj, :],
                            xo_r[:, 2 * b + jl, mj * P:(mj + 1) * P],
                            u_b[b][:, jl, :],
                            start=(jl == 0), stop=(jl == 1))
                # fusedT kts: [wrv(=col0) 0..5 ; wsv(=col1) 6..11]
                nc.vector.tensor_copy(fusedT[:, 0:NMD, b:b + 1],
                                      ps_w[:, :, 0:1])
                nc.scalar.copy(fusedT[:, NMD:2 * NMD, b:b + 1],
                               ps_w[:, :, 1:2])

            # ---- emission ----
            emit_proj("con", DESCALE["con"])
            for b in range(BPC):
                emit_exp("con", b, emit_scores("con", b), INV_SS)

            emit_proj("sup", DESCALE["sup"])
            pts = [emit_exp("sup", b, emit_scores("sup", b), INV_SS)
                   for b in range(BPC)]

            emit_proj("rep", DESCALE["rep"])
            for b in range(BPC):
                emit_u("sup", b, pts[b])
            for b in range(BPC):
                p_r = emit_exp("rep", b, emit_scores("rep", b), INV_SS)
                emit_u("rep", b, p_r)
                emit_vec(b)

            # ---- fuse MLP ----
            ps_h = psum_sm.tile([P, NMD, BPC], fp32, tag="sm", name="ps_h")
            for mh in range(NMD):
                nc.tensor.matmul(ps_h[:, mh, :],
                                 hinitT[0:BPC, mh * P:(mh + 1) * P],
                                 eye4[:], start=True, stop=False)
                for k in range(W1KT):
                    nc.tensor.matmul(ps_h[:, mh, :],
                                     w1[:, k, mh * P:(mh + 1) * P],
                                     fusedT[:, k, :],
                                     start=False, stop=(k == W1KT - 1))
            junk = stats.tile([P, NMD * BPC], fp32, tag="junk")
            nc.scalar.activation(hT[:], ps_h[:], AF.Relu, bias=zbias[:])
            # preload Square+Sqrt ACT LUTs before the layernorm tail
            nc.scalar.activation(junk[0:1, 0:1], eps_t[0:1, :], AF.Square,
                                 bias=eps_t[0:1, :])
            nc.scalar.activation(junk[0:1, 0:1], eps_t[0:1, :], AF.Sqrt,
                                 bias=eps_t[0:1, :])

            ps_y = psum_big.tile([BPC, D], fp32, tag="big", name="ps_y")
            mu_p = stats.tile([BPC, 2], fp32, tag="mu_p")
            sq_p = stats.tile([BPC, 2], fp32, tag="sq_p")
            junk2 = stats.tile([BPC, D], fp32, tag="junk2")
            for ci in range(2):
                cs = slice(ci * 512, min((ci + 1) * 512, D))
                for k in range(NKT):
                    nc.tensor.matmul(ps_y[:, cs], hT[:, k, :], w2[:, k, cs],
                                     start=(k == 0), stop=False)
                nc.tensor.matmul(ps_y[:, cs], ones_row[0:1, 0:BPC],
                                 b2row[0:1, cs], start=False, stop=True)
                nc.vector.reduce_sum(mu_p[:, ci:ci + 1], ps_y[:, cs], axis=AX)
                nc.scalar.activation(junk2[:, cs], ps_y[:, cs], AF.Square,
                                     bias=zbias[0:BPC, :],
                                     accum_out=sq_p[:, ci:ci + 1])

            # ---- layernorm tail: var = E[y^2] - mu^2 ----
            mu = stats.tile([BPC, 1], fp32, tag="mu")
            nc.vector.reduce_sum(mu[:], mu_p[:], axis=AX)
            nc.vector.tensor_scalar_mul(mu[:], mu[:], 1.0 / D)
            msq = stats.tile([BPC, 1], fp32, tag="msq")
            nc.vector.reduce_sum(msq[:], sq_p[:], axis=AX)
            var = stats.tile([BPC, 1], fp32, tag="var")
            nc.vector.tensor_scalar_mul(var[:], msq[:], 1.0 / D)
            mu2 = stats.tile([BPC, 1], fp32, tag="mu2")
            nc.vector.tensor_mul(mu2[:], mu[:], mu[:])
            nc.vector.tensor_sub(var[:], var[:], mu2[:])
            sd = stats.tile([BPC, 1], fp32, tag="sd")
            nc.scalar.activation(sd[:], var[:], AF.Sqrt,
                                 bias=eps_t[0:BPC, :])
            rstd = stats.tile([BPC, 1], fp32, tag="rstd")
            nc.vector.reciprocal(rstd[:], sd[:])
            xc = stats.tile([BPC, D], fp32, tag="xc")
            nc.vector.tensor_scalar(xc[:], ps_y[:], mu[:], None,
                                    op0=mybir.AluOpType.subtract)
            yt = stats.tile([BPC, D], fp32, tag="yt")
            nc.vector.scalar_tensor_tensor(
                yt[:], xc[:], rstd[:], lng[:],
                op0=mybir.AluOpType.mult, op1=mybir.AluOpType.mult)
            nc.vector.tensor_add(yt[:], yt[:], lnb[:])
            nc.sync.dma_start(out=y_out[:], in_=yt[:])

    nc.compile()
    return nc


def _prep_core_inputs(x, x_ids, pad_idx, sep_idx, weights):
    """Host-side packing/quantization/precompute for all cores."""
    false_mask, option_mask = _segment_masks(x_ids, pad_idx, sep_idx)

    (W_anom, b_anom, Wq, bq, Wk, bk, W_fuse1, b_fuse1,
     W_fuse2, b_fuse2, ln_g, ln_b) = weights

    C = {}
    for t in ("sup", "con", "rep"):
        cq = np.concatenate([Wq[t], bq[t][None, :]], axis=0)
        ck = np.concatenate([Wk[t], bk[t][None, :]], axis=0)
        C[t] = (cq @ ck.T) * INV                    # [769, 769]

    global SX_USED, SC_USED, DESCALE, INV_SS       # compile-time constants
    SX_USED = _pow2_scale(np.abs(x).max())
    SC_USED = {t: _pow2_scale(np.abs(C[t][:D, :D]).max())
               for t in ("sup", "con", "rep")}
    DESCALE = {t: float(SG / (SX_USED * SC_USED[t]))
               for t in ("sup", "con", "rep")}
    SS = float(SG * SX_USED)                        # score scale in PSUM
    INV_SS = 1.0 / SS

    def ktile(arr, nkt, dt):
        K, N = arr.shape
        out = np.zeros((nkt * P, N), np.float32)
        out[:K] = arr
        return np.ascontiguousarray(
            out.reshape(nkt, P, N).transpose(1, 0, 2)).astype(dt)

    shared = {}
    for t in ("sup", "con", "rep"):
        shared[f"wc_{t}"] = ktile(C[t][:D, :D] * SC_USED[t], NKT, F8)
    shared["w1"] = ktile(W_fuse1[D:3 * D], W1KT, BF16)
    shared["w2"] = ktile(W_fuse2, NKT, BF16)
    shared["b2row"] = b_fuse2[None, :].astype(BF16)
    shared["eye4"] = np.eye(BPC, dtype=np.float32).astype(BF16)
    shared["lng"] = np.ascontiguousarray(
        np.broadcast_to(ln_g[None, :], (BPC, D)).astype(np.float32))
    shared["lnb"] = np.ascontiguousarray(
        np.broadcast_to(ln_b[None, :], (BPC, D)).astype(np.float32))

    in_maps = []
    for c in range(NCORES):
        xf = np.zeros((BPC, NF, D), np.float32)
        xo = np.zeros((BPC, NO, D), np.float32)
        nfs, nos = [], []
        for i in range(BPC):
            gb = c * BPC + i
            f_idx = np.where(false_mask[gb])[0]
            o_idx = np.where(option_mask[gb])[0]
            xf[i, :len(f_idx)] = x[gb, f_idx]
            xo[i, :len(o_idx)] = x[gb, o_idx]
            nfs.append(len(f_idx))
            nos.append(len(o_idx))

        xf2 = xf.reshape(NQ, D)
        xo2 = xo.reshape(NQ, D)

        # host gate softmax (exact)
        anom = xf2 @ W_anom[:, 0] + b_anom[0]       # [NQ]
        g = np.zeros(NQ, np.float32)
        afv = np.zeros((BPC, D), np.float32)
        for i in range(BPC):
            a = anom[i * NF:i * NF + nfs[i]]
            e = np.exp(a - a.max())
            gi = e / e.sum()
            g[i * NF:i * NF + nfs[i]] = gi
            afv[i] = gi @ xf2[i * NF:i * NF + nfs[i]]

        # con k-bias per q-row; per-column q-bias (+mask) rows
        ccol = xf2 @ C["con"][:D, D]
        mrow = {}
        for t in ("sup", "con", "rep"):
            qb = (xo2 @ C[t][D, :D] + C[t][D, D]).astype(np.float32)  # [NQ]
            if t != "con":
                for i in range(BPC):
                    qb[i * NO + nos[i]:(i + 1) * NO] += NEGM
            # duplicate per jl: layout [b, jl, NO]
            dup = np.repeat(qb.reshape(BPC, 1, NO) * SS, 2, axis=1)
            mrow[t] = dup.reshape(1, BPC * 2 * NO).astype(BF16)

        hinit = afv @ W_fuse1[:D] + b_fuse1         # [BPC, 768]

        m = dict(shared)
        m["xfT8"] = ktile(xf2.T * SX_USED, NKT, F8)
        m["xoT8"] = ktile(xo2.T * SX_USED, NKT, F8)
        m["xo_r"] = np.ascontiguousarray(
            xo2.astype(BF16).reshape(NR, P, D).transpose(1, 0, 2))
        m["g_col"] = np.ascontiguousarray(
            g.reshape(NR, P, 1).transpose(1, 0, 2)).astype(np.float32)
        m["ccol"] = np.ascontiguousarray(
            ccol.reshape(NR, P, 1).transpose(1, 0, 2)).astype(np.float32)
        m["mrow_sup"] = mrow["sup"]
        m["mrow_rep"] = mrow["rep"]
        m["brow_con"] = mrow["con"]
        m["hinitT"] = hinit.astype(BF16)
        in_maps.append(m)
    return in_maps


_CACHED_NC = None
LAST_RESULTS = None
SX_USED = 32.0
SC_USED = {}
DESCALE = {}
INV_SS = 1.0


def kernel(x, x_ids, pad_idx, sep_idx,
           W_anom, b_anom,
           Wq_sup, bq_sup, Wk_sup, bk_sup,
           Wq_con, bq_con, Wk_con, bk_con,
           Wq_rep, bq_rep, Wk_rep, bk_rep,
           W_fuse1, b_fuse1, W_fuse2, b_fuse2,
           ln_g, ln_b):
    from concourse import bass_utils

    global _CACHED_NC, LAST_RESULTS
    x = np.asarray(x, np.float32)
    x_ids = np.asarray(x_ids)
    pad_idx = int(np.asarray(pad_idx))
    sep_idx = int(np.asarray(sep_idx))
    weights = (
        np.asarray(W_anom, np.float32), np.asarray(b_anom, np.float32),
        {"sup": np.asarray(Wq_sup, np.float32),
         "con": np.asarray(Wq_con, np.float32),
         "rep": np.asarray(Wq_rep, np.float32)},
        {"sup": np.asarray(bq_sup, np.float32),
         "con": np.asarray(bq_con, np.float32),
         "rep": np.asarray(bq_rep, np.float32)},
        {"sup": np.asarray(Wk_sup, np.float32),
         "con": np.asarray(Wk_con, np.float32),
         "rep": np.asarray(Wk_rep, np.float32)},
        {"sup": np.asarray(bk_sup, np.float32),
         "con": np.asarray(bk_con, np.float32),
         "rep": np.asarray(bk_rep, np.float32)},
        np.asarray(W_fuse1, np.float32), np.asarray(b_fuse1, np.float32),
        np.asarray(W_fuse2, np.float32), np.asarray(b_fuse2, np.float32),
        np.asarray(ln_g, np.float32), np.asarray(ln_b, np.float32),
    )

    in_maps = _prep_core_inputs(x, x_ids, pad_idx, sep_idx, weights)
    if _CACHED_NC is None:
        _CACHED_NC = _build_program()
    last_err = None
    for attempt in range(3):
        try:
            res = bass_utils.run_bass_kernel_spmd(
                _CACHED_NC, in_maps, list(range(NCORES)))
            break
        except Exception as err:  # transient device-unrecoverable states
            last_err = err
            import time
            time.sleep(5 * (attempt + 1))
            try:
                import jax.extend
                jax.extend.backend.clear_backends()
            except Exception:
                pass
    else:
        raise last_err
    LAST_RESULTS = res
    out = np.zeros((B, D), np.float32)
    for c in range(NCORES):
        out[c * BPC:(c + 1) * BPC] = res.results[c]["y"]
    return out


# revision 12
# speedup vs baseline: 1.5364x; 1.3305x over previous
"""Trainium2 Bass kernel for nn_BertCounterFactCrossOpitionCompetitionTransformer.

v3 — fp8 DoubleRow, host-side gate/bias precompute, contiguous-DMA packing.

Data-parallel over batch (4 batches/core on 8 cores). Per batch the false
(pre-SEP) rows are queries, option (post-SEP) rows are keys; both padded
to 256. Device pipeline per core (q rows packed NQ=4x256=1024):

  G_t   = xf8 @ C8_t            fp8 DoubleRow (C = Wq_aug Wk_aug^T * inv,
                                 host-combined; q-bias row folded into
                                 per-column score bias rows)
  S_t   = G8_t @ xo8^T + brow   fp8 DoubleRow, per-batch 256x256 blocks
  con: tanh_c = tanh(S/SS + c_col)   (c_col = k-bias per q-row, host)
  sup: P = exp(S/SS), rep: P = exp(S/SS + tanh_c); Z row-sums via accum
  u_t   = P_t^T (g/Z)           g = gate softmax, computed exactly on host
  wrv/wsv = xo^T u_t            bf16
  h     = relu(hinit + W1[wrv;wsv] part)   (afv part + bias in hinit, host)
  y     = layernorm(h W2 + b2) [* ln_g + ln_b unless trivial]

All fp8 scales are powers of two; score descale happens inside the
exp/tanh activations. DMA shipped as ~16 large contiguous transfers in
priority order (each dma_start costs ~0.7us on the sync queue engine).
"""

import numpy as np
import ml_dtypes

B, L, D = 32, 512, 768
NCORES = 8
BPC = B // NCORES
NF = 256
NO = 256
NEGM = -30000.0
INV = 1.0 / np.sqrt(D)
P = 128
BF16 = ml_dtypes.bfloat16
F8 = ml_dtypes.float8_e4m3

NKT = 6                 # feature k-tiles (768 = 6*128)
NKP = 3                 # DoubleRow k-tile pairs
NMD = 6                 # m-tiles over 768
NQ = BPC * NF           # 1024 query rows per core
NR = NQ // P            # 8 row-tiles
W1KT = 12               # [wrv; wsv] k-tiles (1536 = 12*128)
SG = 1024.0             # fp8 scale of G


def _segment_masks(x_ids, pad_idx, sep_idx):
    sep_mask = x_ids == sep_idx
    has_sep = sep_mask.any(axis=1)
    idxs = np.argmax(sep_mask.astype(np.int32), axis=1)
    valid_mask = x_ids != pad_idx
    valid_len = valid_mask.sum(axis=1)
    fallback = np.clip(valid_len // 2, 1, max(1, L - 2))
    sep_pos = np.where(has_sep, idxs, fallback)
    pos = np.arange(L)[None, :]
    false_mask = (pos < sep_pos[:, None]) & valid_mask
    option_mask = (pos > sep_pos[:, None]) & valid_mask
    return false_mask, option_mask


def _pow2_scale(absmax, target=224.0):
    return 2.0 ** np.floor(np.log2(target / max(absmax, 1e-30)))


def _build_program():
    import concourse.bacc as bacc
    import concourse.mybir as mybir
    import concourse.tile as tile

    fp32 = mybir.dt.float32
    bf16 = mybir.dt.bfloat16
    fp8 = mybir.dt.float8e4
    AF = mybir.ActivationFunctionType
    DR = mybir.MatmulPerfMode.DoubleRow
    AX = mybir.AxisListType.X

    nc = bacc.Bacc("TRN2", target_bir_lowering=False, debug=False)

    di = {}
    def dram_in(name, shape, dt):
        di[name] = nc.dram_tensor(name, list(shape), dt, kind="ExternalInput")
        return di[name]

    # contiguous per-partition layouts (>=2KB lines per transfer)
    dram_in("xfT8", (P, 2, NKT, 512), fp8)   # c-major feat-tiled false rows
    dram_in("xoT8", (P, 2, NKT, 512), fp8)   # c-major feat-tiled option rows
    for t in ("con", "sup", "rep"):
        dram_in(f"wc_{t}", (P, NMD, NKT, P), fp8)  # m-major C tiles
    dram_in("xo_r", (P, NR, D), bf16)
    dram_in("gccol", (P, NR, 2), fp32)       # [gate | con k-bias]
    dram_in("mrows", (1, 3, BPC * 2 * NO), bf16)  # [sup | rep | con]
    dram_in("w1", (P, W1KT, D), bf16)        # W_fuse1[768:2304] ([wrv; wsv])
    dram_in("w2", (P, NKT, D), bf16)
    dram_in("b2row", (1, D), bf16)
    dram_in("hinitT", (BPC, D), bf16)        # afv@W1[:768]+b1 (host)
    dram_in("eye4", (BPC, BPC), bf16)
    dram_in("lngb", (BPC, 2 * D), fp32)      # [ln_g | ln_b] broadcast
    y_out = nc.dram_tensor("y", [BPC, D], fp32, kind="ExternalOutput")

    with tile.TileContext(nc) as tc:
        with (
            tc.tile_pool(name="const", bufs=1) as const,
            tc.tile_pool(name="xin", bufs=1) as xin,
            tc.tile_pool(name="gt", bufs=1) as gtp,
            tc.tile_pool(name="soft", bufs=2) as soft,
            tc.tile_pool(name="stats", bufs=1) as stats,
            tc.tile_pool(name="psum_big", bufs=2, space="PSUM") as psum_big,
            tc.tile_pool(name="psum_s", bufs=2, space="PSUM") as psum_s,
            tc.tile_pool(name="psum_sm", bufs=2, space="PSUM") as psum_sm,
        ):
            def load(name, shape, dt, chunks=None):
                t_ = xin.tile(list(shape), dt, tag=name, name=f"sb_{name}")
                if chunks is None:
                    nc.sync.dma_start(out=t_[:], in_=di[name][:])
                else:
                    for sl in chunks:
                        nc.sync.dma_start(out=t_[(slice(None),) + sl],
                                          in_=di[name][(slice(None),) + sl])
                return t_

            # ---- DMA priority order (~16 descriptors) ----
            wc = {}
            wc["con"] = load("wc_con", (P, NMD, NKT, P), fp8,
                             chunks=[(slice(0, 3),), (slice(3, 6),)])
            xfT8 = load("xfT8", (P, 2, NKT, 512), fp8,
                        chunks=[(slice(0, 1),), (slice(1, 2),)])
            wc["sup"] = load("wc_sup", (P, NMD, NKT, P), fp8)
            xoT8 = load("xoT8", (P, 2, NKT, 512), fp8)
            wc["rep"] = load("wc_rep", (P, NMD, NKT, P), fp8)
            gccol = load("gccol", (P, NR, 2), fp32)
            mrows = load("mrows", (1, 3, BPC * 2 * NO), bf16)
            xo_r = load("xo_r", (P, NR, D), bf16)
            w1 = load("w1", (P, W1KT, D), bf16)
            w2 = load("w2", (P, NKT, D), bf16)
            b2row = load("b2row", (1, D), bf16)
            hinitT = load("hinitT", (BPC, D), bf16)
            eye4 = load("eye4", (BPC, BPC), bf16)
            lngb = load("lngb", (BPC, 2 * D), fp32)

            MIDX = {"sup": 0, "rep": 1, "con": 2}

            ones_row = const.tile([1, P], bf16, tag="ones_row")
            nc.vector.memset(ones_row[:], 1.0)
            zbias = const.tile([P, 1], fp32, tag="zbias")
            nc.vector.memset(zbias[:], 0.0)
            eps_t = const.tile([P, 1], fp32, tag="eps")
            nc.vector.memset(eps_t[:], 1e-5)
            junk1 = const.tile([1, 4], fp32, tag="junk1")
            # preload all ACT LUTs while DMAs stream (scalar engine is idle)
            for f in (AF.Tanh, AF.Exp, AF.Square, AF.Sqrt):
                nc.scalar.activation(junk1[0:1, 0:1], eps_t[0:1, :], f,
                                     bias=eps_t[0:1, :])

            # persistent tiles
            GT8 = {t: gtp.tile([P, NKT, NQ], fp8, tag=f"GT_{t}",
                               name=f"GT_{t}") for t in ("con", "sup", "rep")}
            Zs = {t: stats.tile([P, NR, 1], fp32, tag=f"Z_{t}", name=f"Z_{t}")
                  for t in ("sup", "rep")}
            tanh_c = [stats.tile([P, 2, NO], fp32, tag=f"tanh{b}",
                                 name=f"tanh{b}") for b in range(BPC)]
            pts_sup = [stats.tile([P, 2, NO], bf16, tag=f"psup{b}",
                                  name=f"psup{b}") for b in range(BPC)]
            u_b = [stats.tile([P, 2, 2], bf16, tag=f"u{b}", name=f"u{b}")
                   for b in range(BPC)]
            fusedT = stats.tile([P, W1KT, BPC], bf16, tag="fusedT")
            hT = stats.tile([P, NMD, BPC], bf16, tag="hT")

            def emit_proj(t, gdescale):
                w_ = wc[t]
                for m in range(NMD):
                    ps = psum_big.tile([P, NQ], fp32, tag="big",
                                       name=f"ps_p{t}{m}")
                    for c in range(2):
                        for k in range(NKP):
                            nc.tensor.matmul(
                                ps[:, c * 512:(c + 1) * 512],
                                w_[:, m, 2 * k:2 * k + 2, :],
                                xfT8[:, c, 2 * k:2 * k + 2, :],
                                start=(k == 0), stop=(k == NKP - 1),
                                perf_mode=DR)
                    # fp8 quantize G with pow2 rescale (vector; gpsimd
                    # cannot read PSUM, scalar must stay free for exp/tanh)
                    nc.vector.tensor_scalar_mul(GT8[t][:, m, :], ps[:],
                                                gdescale)

            def emit_scores(t, b):
                ps_s = psum_s.tile([P, 2, NO], fp32, tag="s",
                                   name=f"ps_s{t}{b}")
                mi = MIDX[t]
                for jl in range(2):
                    q0 = b * NF + jl * P
                    for k in range(NKP):
                        nc.tensor.matmul(
                            ps_s[:, jl, :],
                            GT8[t][:, 2 * k:2 * k + 2, q0:q0 + P],
                            xoT8[:, b // 2, 2 * k:2 * k + 2,
                                 (b % 2) * NO:(b % 2 + 1) * NO],
                            start=(k == 0), stop=False, perf_mode=DR)
                    o0 = (b * 2 + jl) * NO
                    nc.tensor.matmul(
                        ps_s[:, jl, :], ones_row[0:1, 0:P],
                        mrows[0:1, mi, o0:o0 + NO],
                        start=False, stop=True)
                return ps_s

            def emit_exp(t, b, ps_s, inv_ss):
                if t == "con":
                    for jl in range(2):
                        nc.scalar.activation(tanh_c[b][:, jl, :],
                                             ps_s[:, jl, :], AF.Tanh,
                                             scale=inv_ss,
                                             bias=gccol[:, 2 * b + jl, 1:2])
                    return None
                if t == "rep":
                    a_t = soft.tile([P, 2, NO], fp32, tag="A", name=f"A{b}")
                    nc.vector.scalar_tensor_tensor(
                        a_t[:], ps_s[:], inv_ss, tanh_c[b][:],
                        op0=mybir.AluOpType.mult, op1=mybir.AluOpType.add)
                    p_t = soft.tile([P, 2, NO], bf16, tag="Pr", name=f"Pr{b}")
                    for jl in range(2):
                        nc.scalar.activation(p_t[:, jl, :], a_t[:, jl, :],
                                             AF.Exp, bias=zbias[:],
                                             accum_out=Zs[t][:, 2 * b + jl, :])
                    return p_t
                p_t = pts_sup[b]
                for jl in range(2):
                    nc.scalar.activation(p_t[:, jl, :], ps_s[:, jl, :],
                                         AF.Exp, scale=inv_ss, bias=zbias[:],
                                         accum_out=Zs[t][:, 2 * b + jl, :])
                return p_t

            def emit_u(t, b, p_t):
                rz = soft.tile([P, 2, 1], fp32, tag="rz", name=f"rz{t}{b}")
                nc.vector.reciprocal(rz[:], Zs[t][:, 2 * b:2 * b + 2, :])
                w_t = soft.tile([P, 2, 1], bf16, tag="w", name=f"w{t}{b}")
                nc.vector.tensor_mul(w_t[:], gccol[:, 2 * b:2 * b + 2, 0:1],
                                     rz[:])
                ps_u = psum_sm.tile([P, 2, 1], fp32, tag="sm",
                                    name=f"ps_u{t}{b}")
                for mo_t in range(2):
                    for jl in range(2):
                        nc.tensor.matmul(
                            ps_u[:, mo_t, :],
                            p_t[:, jl, mo_t * P:(mo_t + 1) * P],
                            w_t[:, jl, :],
                            start=(jl == 0), stop=(jl == 1))
                tcol = 0 if t == "rep" else 1
                nc.vector.tensor_copy(u_b[b][:, :, tcol:tcol + 1], ps_u[:])

            def emit_vec(b):
                ps_w = psum_sm.tile([P, NMD, 2], fp32, tag="sm",
                                    name=f"ps_w{b}")
                for mj in range(NMD):
                    for jl in range(2):
                        nc.tensor.matmul(
                            ps_w[:, mj, :],
                            xo_r[:, 2 * b + jl, mj * P:(mj + 1) * P],
                            u_b[b][:, jl, :],
                            start=(jl == 0), stop=(jl == 1))
                # fusedT kts: [wrv(=col0) 0..5 ; wsv(=col1) 6..11]
                nc.vector.tensor_copy(fusedT[:, 0:NMD, b:b + 1],
                                      ps_w[:, :, 0:1])
                nc.vector.tensor_copy(fusedT[:, NMD:2 * NMD, b:b + 1],
                                      ps_w[:, :, 1:2])

            # ---- emission ----
            emit_proj("con", DESCALE["con"])
            for b in range(BPC):
                emit_exp("con", b, emit_scores("con", b), INV_SS)

            emit_proj("sup", DESCALE["sup"])
            pts = [emit_exp("sup", b, emit_scores("sup", b), INV_SS)
                   for b in range(BPC)]

            emit_proj("rep", DESCALE["rep"])
            for b in range(BPC):
                emit_u("sup", b, pts[b])
            for b in range(BPC):
                p_r = emit_exp("rep", b, emit_scores("rep", b), INV_SS)
                emit_u("rep", b, p_r)
                emit_vec(b)

            # ---- fuse MLP ----
            ps_h = psum_sm.tile([P, NMD, BPC], fp32, tag="sm", name="ps_h")
            for mh in range(NMD):
                nc.tensor.matmul(ps_h[:, mh, :],
                                 hinitT[0:BPC, mh * P:(mh + 1) * P],
                                 eye4[:], start=True, stop=False)
                for k in range(W1KT):
                    nc.tensor.matmul(ps_h[:, mh, :],
                                     w1[:, k, mh * P:(mh + 1) * P],
                                     fusedT[:, k, :],
                                     start=False, stop=(k == W1KT - 1))
            nc.scalar.activation(hT[:], ps_h[:], AF.Relu, bias=zbias[:])

            ps_y = psum_big.tile([BPC, D], fp32, tag="big", name="ps_y")
            mu_p = stats.tile([BPC, 2], fp32, tag="mu_p")
            sq_p = stats.tile([BPC, 2], fp32, tag="sq_p")
            junk2 = stats.tile([BPC, D], fp32, tag="junk2")
            for ci in range(2):
                cs = slice(ci * 512, min((ci + 1) * 512, D))
                for k in range(NKT):
                    nc.tensor.matmul(ps_y[:, cs], hT[:, k, :], w2[:, k, cs],
                                     start=(k == 0), stop=False)
                nc.tensor.matmul(ps_y[:, cs], ones_row[0:1, 0:BPC],
                                 b2row[0:1, cs], start=False, stop=True)
                nc.vector.reduce_sum(mu_p[:, ci:ci + 1], ps_y[:, cs], axis=AX)
                nc.scalar.activation(junk2[:, cs], ps_y[:, cs], AF.Square,
                                     bias=zbias[0:BPC, :],
                                     accum_out=sq_p[:, ci:ci + 1])

            # ---- layernorm tail: var = E[y^2] - mu^2 ----
            mu = stats.tile([BPC, 1], fp32, tag="mu")
            nc.vector.reduce_sum(mu[:], mu_p[:], axis=AX)
            nc.vector.tensor_scalar_mul(mu[:], mu[:], 1.0 / D)
            var = stats.tile([BPC, 1], fp32, tag="var")
            nc.vector.reduce_sum(var[:], sq_p[:], axis=AX)
            nc.vector.tensor_scalar_mul(var[:], var[:], 1.0 / D)
            mu2 = stats.tile([BPC, 1], fp32, tag="mu2")
            nc.vector.tensor_mul(mu2[:], mu[:], mu[:])
            nc.vector.tensor_sub(var[:], var[:], mu2[:])
            sd = stats.tile([BPC, 1], fp32, tag="sd")
            nc.scalar.activation(sd[:], var[:], AF.Sqrt,
                                 bias=eps_t[0:BPC, :])
            rstd = stats.tile([BPC, 1], fp32, tag="rstd")
            nc.vector.reciprocal(rstd[:], sd[:])
            yt = stats.tile([BPC, D], fp32, tag="yt")
            nc.vector.tensor_scalar(yt[:], ps_y[:], mu[:], rstd[:],
                                    op0=mybir.AluOpType.subtract,
                                    op1=mybir.AluOpType.mult)
            if not LN_TRIVIAL:
                nc.vector.tensor_mul(yt[:], yt[:], lngb[:, 0:D])
                nc.vector.tensor_add(yt[:], yt[:], lngb[:, D:2 * D])
            nc.sync.dma_start(out=y_out[:], in_=yt[:])

    nc.compile()
    return nc


def _prep_core_inputs(x, x_ids, pad_idx, sep_idx, weights):
    """Host-side packing/quantization/precompute for all cores."""
    false_mask, option_mask = _segment_masks(x_ids, pad_idx, sep_idx)

    (W_anom, b_anom, Wq, bq, Wk, bk, W_fuse1, b_fuse1,
     W_fuse2, b_fuse2, ln_g, ln_b) = weights

    C = {}
    for t in ("sup", "con", "rep"):
        cq = np.concatenate([Wq[t], bq[t][None, :]], axis=0)
        ck = np.concatenate([Wk[t], bk[t][None, :]], axis=0)
        C[t] = (cq @ ck.T) * INV                    # [769, 769]

    global SX_USED, SC_USED, DESCALE, INV_SS, LN_TRIVIAL
    SX_USED = _pow2_scale(np.abs(x).max())
    SC_USED = {t: _pow2_scale(np.abs(C[t][:D, :D]).max())
               for t in ("sup", "con", "rep")}
    DESCALE = {t: float(SG / (SX_USED * SC_USED[t]))
               for t in ("sup", "con", "rep")}
    SS = float(SG * SX_USED)                        # score scale in PSUM
    INV_SS = 1.0 / SS
    LN_TRIVIAL = bool(np.all(ln_g == 1.0) and np.all(ln_b == 0.0))

    def ktile(arr, nkt, dt):
        K, N = arr.shape
        out = np.zeros((nkt * P, N), np.float32)
        out[:K] = arr
        return np.ascontiguousarray(
            out.reshape(nkt, P, N).transpose(1, 0, 2)).astype(dt)

    def cmajor(arr, dt):
        # [768, 1024] -> [P, 2, NKT, 512]: [p, c, kt, col]
        t_ = arr.reshape(NKT, P, 2, 512).transpose(1, 2, 0, 3)
        return np.ascontiguousarray(t_).astype(dt)

    def mmajor(arr, dt):
        # [768, 768] -> [P, NMD, NKT, P]: [p, m, kt, mcol]
        t_ = arr.reshape(NKT, P, NMD, P).transpose(1, 2, 0, 3)
        return np.ascontiguousarray(t_).astype(dt)

    shared = {}
    for t in ("sup", "con", "rep"):
        shared[f"wc_{t}"] = mmajor(C[t][:D, :D] * SC_USED[t], F8)
    shared["w1"] = ktile(W_fuse1[D:3 * D], W1KT, BF16)
    shared["w2"] = ktile(W_fuse2, NKT, BF16)
    shared["b2row"] = b_fuse2[None, :].astype(BF16)
    shared["eye4"] = np.eye(BPC, dtype=np.float32).astype(BF16)
    shared["lngb"] = np.ascontiguousarray(np.broadcast_to(
        np.concatenate([ln_g, ln_b])[None, :], (BPC, 2 * D))).astype(
            np.float32)

    in_maps = []
    for c in range(NCORES):
        xf = np.zeros((BPC, NF, D), np.float32)
        xo = np.zeros((BPC, NO, D), np.float32)
        nfs, nos = [], []
        for i in range(BPC):
            gb = c * BPC + i
            f_idx = np.where(false_mask[gb])[0]
            o_idx = np.where(option_mask[gb])[0]
            xf[i, :len(f_idx)] = x[gb, f_idx]
            xo[i, :len(o_idx)] = x[gb, o_idx]
            nfs.append(len(f_idx))
            nos.append(len(o_idx))

        xf2 = xf.reshape(NQ, D)
        xo2 = xo.reshape(NQ, D)

        # host gate softmax (exact)
        anom = xf2 @ W_anom[:, 0] + b_anom[0]       # [NQ]
        g = np.zeros(NQ, np.float32)
        afv = np.zeros((BPC, D), np.float32)
        for i in range(BPC):
            a = anom[i * NF:i * NF + nfs[i]]
            e = np.exp(a - a.max())
            gi = e / e.sum()
            g[i * NF:i * NF + nfs[i]] = gi
            afv[i] = gi @ xf2[i * NF:i * NF + nfs[i]]

        # con k-bias per q-row; per-column q-bias (+mask) rows
        ccol = xf2 @ C["con"][:D, D]
        mr = np.zeros((3, BPC, 2, NO), np.float32)
        for t in ("sup", "con", "rep"):
            qb = (xo2 @ C[t][D, :D] + C[t][D, D]).astype(np.float32)
            if t != "con":
                for i in range(BPC):
                    qb[i * NO + nos[i]:(i + 1) * NO] += NEGM
            mr[{"sup": 0, "rep": 1, "con": 2}[t]] = \
                np.repeat(qb.reshape(BPC, 1, NO) * SS, 2, axis=1)

        hinit = afv @ W_fuse1[:D] + b_fuse1         # [BPC, 768]

        gc = np.stack([g, ccol], axis=1)            # [NQ, 2]

        m = dict(shared)
        m["xfT8"] = cmajor(xf2.T * SX_USED, F8)
        m["xoT8"] = cmajor(xo2.T * SX_USED, F8)
        m["xo_r"] = np.ascontiguousarray(
            xo2.astype(BF16).reshape(NR, P, D).transpose(1, 0, 2))
        m["gccol"] = np.ascontiguousarray(
            gc.reshape(NR, P, 2).transpose(1, 0, 2)).astype(np.float32)
        m["mrows"] = mr.reshape(1, 3, BPC * 2 * NO).astype(BF16)
        m["hinitT"] = hinit.astype(BF16)
        in_maps.append(m)
    return in_maps


_CACHED_NC = None
LAST_RESULTS = None
SX_USED = 32.0
SC_USED = {}
DESCALE = {}
INV_SS = 1.0
LN_TRIVIAL = True


def kernel(x, x_ids, pad_idx, sep_idx,
           W_anom, b_anom,
           Wq_sup, bq_sup, Wk_sup, bk_sup,
           Wq_con, bq_con, Wk_con, bk_con,
           Wq_rep, bq_rep, Wk_rep, bk_rep,
           W_fuse1, b_fuse1, W_fuse2, b_fuse2,
           ln_g, ln_b):
    from concourse import bass_utils

    global _CACHED_NC, LAST_RESULTS
    x = np.asarray(x, np.float32)
    x_ids = np.asarray(x_ids)
    pad_idx = int(np.asarray(pad_idx))
    sep_idx = int(np.asarray(sep_idx))
    weights = (
        np.asarray(W_anom, np.float32), np.asarray(b_anom, np.float32),
        {"sup": np.asarray(Wq_sup, np.float32),
         "con": np.asarray(Wq_con, np.float32),
         "rep": np.asarray(Wq_rep, np.float32)},
        {"sup": np.asarray(bq_sup, np.float32),
         "con": np.asarray(bq_con, np.float32),
         "rep": np.asarray(bq_rep, np.float32)},
        {"sup": np.asarray(Wk_sup, np.float32),
         "con": np.asarray(Wk_con, np.float32),
         "rep": np.asarray(Wk_rep, np.float32)},
        {"sup": np.asarray(bk_sup, np.float32),
         "con": np.asarray(bk_con, np.float32),
         "rep": np.asarray(bk_rep, np.float32)},
        np.asarray(W_fuse1, np.float32), np.asarray(b_fuse1, np.float32),
        np.asarray(W_fuse2, np.float32), np.asarray(b_fuse2, np.float32),
        np.asarray(ln_g, np.float32), np.asarray(ln_b, np.float32),
    )

    in_maps = _prep_core_inputs(x, x_ids, pad_idx, sep_idx, weights)
    if _CACHED_NC is None:
        _CACHED_NC = _build_program()
    last_err = None
    for attempt in range(3):
        try:
            res = bass_utils.run_bass_kernel_spmd(
                _CACHED_NC, in_maps, list(range(NCORES)))
            break
        except Exception as err:  # transient device-unrecoverable states
            last_err = err
            import time
            time.sleep(5 * (attempt + 1))
            try:
                import jax.extend
                jax.extend.backend.clear_backends()
            except Exception:
                pass
    else:
        raise last_err
    LAST_RESULTS = res
    out = np.zeros((B, D), np.float32)
    for c in range(NCORES):
        out[c * BPC:(c + 1) * BPC] = res.results[c]["y"]
    return out
